# revision 1
# baseline (speedup 1.0000x reference)
"""Trainium2 Bass kernel for BottleneckAttention.

Reference computation (per sample b):
  xf = x[b] reshaped [C, N]                        C=256, N=4096
  q = Wq @ xf + bq          [32, N]
  k = Wk @ xf + bk          [32, N]
  v = Wv @ xf + bv          [C, N]
  att = softmax_j(q_i . k_j / sqrt(32))            [N, N]
  out[c, i] = sum_j v[c, j] att[i, j]
  fused = Wf @ concat([gamma*out, x]) + bf         [C, N]

Sharding: 8 cores = 4 samples x 2 query-halves (each core owns 2048 query
positions i of one sample, and computes k/v for all 4096 key positions of
that sample). No cross-core communication.

Per-core dataflow (all SPMD-identical; per-core data differs via in_maps):
  - q_rep [128, 2048] bf16: q replicated 4x along partitions (tiled weights)
  - k_rep [128, 4096] bf16: k replicated 4x along partitions
  - vt    [4096, 257]  bf16: v transposed (j on partitions, 32 tiles of 128)
          with a ones-column at c=256 (gives sumexp during the AV matmul)
  - main loop over 8 i-blocks (256 wide) x 8 j-groups (4 j-tiles each):
      attT[j, i] = k_rep-slice.T @ q_rep-slice, two concurrent row-group
        matmul pairs (tile_position 0/32), pairs draining to distinct PSUM
        banks (concurrent same-bank drains crash the PE)
      expT = exp(attT / sqrt(32))  (ScalarE, bf16 out, one [128,1024] call)
      o'[i, 0:257] += expT-slice.T @ vt-slice   (PSUM, software-pipelined
        one j-group behind exp so PE/ACT overlap)
    vt generation is emitted just-in-time inside the first i-block; a burst
    of scratch matmuls at t~0 opens the HAM clock gate before real work.
  - tail: normalize o' by 1/sumexp, TensorE-transpose to oT [c, i], then
    fused = WfxT-chunks @ x-half (fp32) + (gamma*Wfo)T-chunks @ oT (bf16)
    + bf_eff, where bf_eff = bf + gamma * Wf[:, :256] @ bv (host-folded).

The attention branch contributes only ~5e-5 of the output norm (gamma=0.01
and the x-passthrough dominates), so bf16 there is far below tolerance; the
x-passthrough runs in fp32 (rel err ~2e-7 overall).
"""

import numpy as np
import ml_dtypes
from contextlib import ExitStack

import concourse.bass as bass
import concourse.tile as tile
from concourse import bacc, mybir
from concourse.bass_utils import run_bass_kernel_spmd

B, C, CK, H, W = 4, 256, 32, 64, 64
N = H * W            # 4096
NH = N // 2          # 2048 query positions per core
NCORES = 8
NJT = N // 128       # 32 j-tiles
VTP = C + 1          # 257: o_raw row width (sumexp column at 256)
VTP8 = 272           # fp8 vt row pitch (16B-aligned for DoubleRow)
SCALE = float(1.0 / np.sqrt(np.float32(CK)))

BF16 = mybir.dt.bfloat16
F32 = mybir.dt.float32
F32R = mybir.dt.float32r
FP8 = mybir.dt.float8e4
X_DT = F32           # dtype for the precision-critical x-path matmuls
NP_BF16 = ml_dtypes.bfloat16

_CACHE = {}


def ts(i, size):
    return bass.ts(i, size)




def make_tc_tile(pool):
    def tc_tile(tc, shape, dtype, name):
        return pool.tile(shape, dtype, tag=name, name=name)
    return tc_tile

def _build_nc():
    nc = bacc.Bacc("TRN2", target_bir_lowering=False, debug=False,
                   num_devices=NCORES)

    # ---- DRAM I/O ----------------------------------------------------------
    d_xf16 = nc.dram_tensor("xf16", [C, N], BF16, kind="ExternalInput").ap()
    d_xh32 = nc.dram_tensor("xh32", [C, NH], X_DT, kind="ExternalInput").ap()
    d_wq4 = nc.dram_tensor("wq4", [C, 128], BF16, kind="ExternalInput").ap()
    d_wk4 = nc.dram_tensor("wk4", [C, 128], BF16, kind="ExternalInput").ap()
    d_wv = nc.dram_tensor("wv", [C, C], BF16, kind="ExternalInput").ap()
    d_wfo = nc.dram_tensor("wfo", [C, C], BF16, kind="ExternalInput").ap()
    d_wfx = nc.dram_tensor("wfx", [C, C], X_DT, kind="ExternalInput").ap()
    d_bq4 = nc.dram_tensor("bq4", [128, 1], F32, kind="ExternalInput").ap()
    d_bk4 = nc.dram_tensor("bk4", [128, 1], F32, kind="ExternalInput").ap()
    d_bfe = nc.dram_tensor("bfe", [C, 1], F32, kind="ExternalInput").ap()
    d_ident = nc.dram_tensor("ident", [128, 128], BF16, kind="ExternalInput").ap()
    d_out = nc.dram_tensor("out", [C, NH], F32, kind="ExternalOutput").ap()

    with tile.TileContext(nc) as tc, ExitStack() as ctx:
        # ---- persistent SBUF tensors --------------------------------------
        const_pool = ctx.enter_context(tc.tile_pool(name="const_pool", bufs=1))
        tc_tile = make_tc_tile(const_pool)
        xf16_s = [tc_tile(tc, [128, N], BF16, name=f"xf16_{c}") for c in range(2)]
        xh32_s = [tc_tile(tc, [128, NH], X_DT, name=f"xh32_{c}") for c in range(2)]
        wq4_s = [tc_tile(tc, [128, 128], BF16, name=f"wq4_{c}") for c in range(2)]
        wk4_s = [tc_tile(tc, [128, 128], BF16, name=f"wk4_{c}") for c in range(2)]
        wv_s = [tc_tile(tc, [128, C], BF16, name=f"wv_{c}") for c in range(2)]
        wfo_s = [tc_tile(tc, [128, C], BF16, name=f"wfo_{c}") for c in range(2)]
        wfx_s = [tc_tile(tc, [128, C], X_DT, name=f"wfx_{c}") for c in range(2)]
        bq4_s = tc_tile(tc, [128, 1], F32, name="bq4_s")
        bk4_s = tc_tile(tc, [128, 1], F32, name="bk4_s")
        bfe_s = [tc_tile(tc, [128, 1], F32, name=f"bfe_{c}") for c in range(2)]
        ident_s = tc_tile(tc, [128, 128], BF16, name="ident_s")
        q_rep = tc_tile(tc, [128, NH], BF16, name="q_rep")
        k_rep = tc_tile(tc, [128, N], BF16, name="k_rep")
        vt_all = tc_tile(tc, [128, NJT * VTP], BF16, name="vt_all")
        ot_s = [tc_tile(tc, [128, NH], BF16, name=f"ot_{c}") for c in range(2)]
        onorm_all = tc_tile(tc, [128, NH * 2], BF16, name="onorm_all")
        oraw_all = tc_tile(tc, [128, 16 * VTP], F32, name="oraw_all")

        # ---- pools --------------------------------------------------------
        ps_small = ctx.enter_context(
            tc.tile_pool(name="ps_small", bufs=4, space="PSUM"))
        ps_big = ctx.enter_context(
            tc.tile_pool(name="ps_big", bufs=2, space="PSUM"))
        exp_pool = ctx.enter_context(tc.tile_pool(name="exp_pool", bufs=3))
        onorm_pool = ctx.enter_context(tc.tile_pool(name="onorm_pool", bufs=2))
        rec_pool = ctx.enter_context(tc.tile_pool(name="rec_pool", bufs=2))
        fo_pool = ctx.enter_context(tc.tile_pool(name="fo_pool", bufs=3))

        # ---- phase 0: load everything -------------------------------------
        # split loads across the sync (HWDGE) and gpsimd (SWDGE) queues,
        # ordered so the q projection (cols 0:NH of xf16) unblocks first.
        nc.sync.dma_start(wq4_s[0][:], d_wq4[ts(0, 128), :])
        nc.sync.dma_start(wq4_s[1][:], d_wq4[ts(1, 128), :])
        nc.sync.dma_start(bq4_s[:], d_bq4[:])
        nc.sync.dma_start(xf16_s[0][:, 0:NH], d_xf16[ts(0, 128), 0:NH])
        nc.sync.dma_start(wk4_s[0][:], d_wk4[ts(0, 128), :])
        nc.sync.dma_start(wk4_s[1][:], d_wk4[ts(1, 128), :])
        nc.sync.dma_start(bk4_s[:], d_bk4[:])
        nc.sync.dma_start(xf16_s[0][:, NH:N], d_xf16[ts(0, 128), NH:N])
        nc.sync.dma_start(wv_s[0][:], d_wv[ts(0, 128), :])
        nc.sync.dma_start(ident_s[:], d_ident[:])
        nc.sync.dma_start(wfo_s[0][:], d_wfo[ts(0, 128), :])
        nc.sync.dma_start(wfx_s[0][:], d_wfx[ts(0, 128), :])
        nc.sync.dma_start(bfe_s[0][:], d_bfe[ts(0, 128), :])
        nc.sync.dma_start(xh32_s[0][:], d_xh32[ts(0, 128), :])
        nc.gpsimd.dma_start(xf16_s[1][:, 0:NH], d_xf16[ts(1, 128), 0:NH])
        nc.gpsimd.dma_start(xf16_s[1][:, NH:N], d_xf16[ts(1, 128), NH:N])
        nc.gpsimd.dma_start(wv_s[1][:], d_wv[ts(1, 128), :])
        nc.gpsimd.dma_start(wfo_s[1][:], d_wfo[ts(1, 128), :])
        nc.gpsimd.dma_start(wfx_s[1][:], d_wfx[ts(1, 128), :])
        nc.gpsimd.dma_start(bfe_s[1][:], d_bfe[ts(1, 128), :])
        nc.gpsimd.dma_start(xh32_s[1][:], d_xh32[ts(1, 128), :])

        # ---- phase 0.5: PE warmup ----------------------------------------
        # Dependency-free matmuls on scratch data keep TensorE busy from t~0
        # so the HAM clock gate opens (2.4GHz) before real work arrives.
        warm_src = tc_tile(tc, [128, 128], BF16, name="warm_src")
        nc.vector.memset(warm_src[:], 0.25)
        for w in range(30):
            wp = ps_big.tile([128, 128], F32, tag="att", name="warm_ps")
            nc.tensor.matmul(wp[:], lhsT=warm_src[:], rhs=warm_src[:, 0:128],
                             start=True, stop=True)

        # ---- phase 1: projections ----------------------------------------
        # Emission order = DVE FIFO order: the first attT group only needs
        # q chunk 0 and k chunks 0-1, so emit those copies first.
        def emit_q(n):
            qp = ps_small.tile([128, 512], F32, tag="ps_s", name="qp")
            nc.tensor.matmul(qp[:], lhsT=wq4_s[0][:],
                             rhs=xf16_s[0][:, ts(n, 512)], start=True, stop=False)
            nc.tensor.matmul(qp[:], lhsT=wq4_s[1][:],
                             rhs=xf16_s[1][:, ts(n, 512)], start=False, stop=True)
            nc.vector.tensor_scalar(q_rep[:, ts(n, 512)], qp[:], bq4_s[:], None,
                                    op0=mybir.AluOpType.add)

        def emit_k(n):
            kp = ps_small.tile([128, 512], F32, tag="ps_s", name="kp")
            nc.tensor.matmul(kp[:], lhsT=wk4_s[0][:],
                             rhs=xf16_s[0][:, ts(n, 512)], start=True, stop=False)
            nc.tensor.matmul(kp[:], lhsT=wk4_s[1][:],
                             rhs=xf16_s[1][:, ts(n, 512)], start=False, stop=True)
            nc.vector.tensor_scalar(k_rep[:, ts(n, 512)], kp[:], bk4_s[:], None,
                                    op0=mybir.AluOpType.add)

        emit_q(0)
        emit_k(0)
        emit_k(1)
        for n in range(1, NH // 512):
            emit_q(n)
        for n in range(2, N // 512):
            emit_k(n)

        # ones column of vt (index C of each j-tile slice)
        for jt in range(NJT):
            nc.vector.memset(vt_all[:, jt * VTP + C: jt * VTP + C + 1], 1.0)

        # vt [j, c]: lhsT = xf16 j-slice (weights), rhs = WvT chunk.
        # Tiles 0..7 up front; the rest are emitted just-in-time inside the
        # first i-block so the softmax pipeline starts ~10us earlier.
        def emit_vt(jt):
            vp = ps_small.tile([128, C], F32, tag="ps_s", name="vp")
            nc.tensor.matmul(vp[:], lhsT=xf16_s[0][:, ts(jt, 128)],
                             rhs=wv_s[0][:], start=True, stop=False)
            nc.tensor.matmul(vp[:], lhsT=xf16_s[1][:, ts(jt, 128)],
                             rhs=wv_s[1][:], start=False, stop=True)
            nc.vector.tensor_copy(vt_all[:, jt * VTP: jt * VTP + C], vp[:])

        for jt in range(4):
            emit_vt(jt)

        # ---- phase 2: attention main loop --------------------------------
        # 8 i-blocks of 256 positions; 8 j-groups of 4 j-tiles, 4-way
        # row-packed attT (k replicas at partition offsets 0/32/64/96).
        NIB = NH // 256          # 8
        NJG = NJT // 4           # 8
        for ib in range(NIB):
            o_ps = [ps_small.tile([128, VTP], F32, tag="ps_s", name=f"o_ps{it}")
                    for it in range(2)]
            pend_av = None
            for jg in range(NJG):
                att_ps = ps_big.tile([128, 1024], F32, tag="att", name="att_ps")
                # concurrent row-group pairs must drain into different
                # PSUM banks: s=0/2 -> bank0 (cols 0/256), s=1/3 -> bank1.
                COLOF = (0, 512, 256, 768)
                for s in range(4):
                    jt = 4 * jg + s
                    g = 32 * (s % 2)
                    nc.tensor.matmul(
                        att_ps[:, COLOF[s]: COLOF[s] + 256],
                        lhsT=k_rep[g: g + 32, ts(jt, 128)],
                        rhs=q_rep[g: g + 32, ts(ib, 256)],
                        start=True, stop=True, tile_position=(g, 0))
                expt = exp_pool.tile([128, 1024], BF16, tag="expt", name="expt")
                nc.scalar.activation(expt[:], att_ps[:],
                                     mybir.ActivationFunctionType.Exp,
                                     scale=SCALE)
                if ib == 0 and jg < 7:
                    for v4 in range(4):
                        emit_vt(4 + 4 * jg + v4)
                if pend_av is not None:
                    pend_av()

                def make_av(expt=expt, jg=jg, o_ps=o_ps):
                    def emit():
                        COLOF = (0, 512, 256, 768)
                        for s in range(4):
                            jt = 4 * jg + s
                            for it in range(2):
                                nc.tensor.matmul(
                                    o_ps[it][:],
                                    lhsT=expt[:, COLOF[s] + 128 * it:
                                              COLOF[s] + 128 * (it + 1)],
                                    rhs=vt_all[:, jt * VTP: (jt + 1) * VTP],
                                    start=(jt == 0), stop=(jt == NJT - 1))
                    return emit
                pend_av = make_av()
            pend_av()

            # evacuate raw o' (with its sumexp column) to SBUF; normalize and
            # transpose happen in the tail phase.
            for it in range(2):
                gi = 2 * ib + it
                nc.vector.tensor_copy(oraw_all[:, VTP * gi: VTP * (gi + 1)],
                                      o_ps[it][:, 0:VTP])

        # ---- phase 3: normalize, transpose o, fused output projection ----
        for gi in range(16):
            rec = rec_pool.tile([128, 1], F32, tag="rec", name="rec")
            nc.vector.reciprocal(rec[:],
                                 oraw_all[:, VTP * gi + C: VTP * gi + C + 1])
            nc.vector.tensor_scalar(onorm_all[:, C * gi: C * (gi + 1)],
                                    oraw_all[:, VTP * gi: VTP * gi + C],
                                    rec[:], None, op0=mybir.AluOpType.mult)

        def emit_tr(n):
            for it4 in range(4):
                gi = 4 * n + it4
                for ch in range(2):
                    tp = ps_small.tile([128, 128], BF16, tag="ps_s", name="tp")
                    nc.tensor.transpose(
                        tp[:],
                        onorm_all[:, C * gi + 128 * ch: C * gi + 128 * (ch + 1)],
                        ident_s[:])
                    nc.vector.tensor_copy(
                        ot_s[ch][:, 128 * gi: 128 * (gi + 1)], tp[:])

        emit_tr(0)
        for n in range(NH // 512):
            if n + 1 < NH // 512:
                emit_tr(n + 1)
            for fh in range(2):
                fp = ps_small.tile([128, 512], F32, tag="ps_s", name="fp")
                nc.tensor.matmul(fp[:], lhsT=wfx_s[0][:, ts(fh, 128)],
                                 rhs=xh32_s[0][:, ts(n, 512)],
                                 start=True, stop=False)
                nc.tensor.matmul(fp[:], lhsT=wfx_s[1][:, ts(fh, 128)],
                                 rhs=xh32_s[1][:, ts(n, 512)],
                                 start=False, stop=False)
                nc.tensor.matmul(fp[:], lhsT=wfo_s[0][:, ts(fh, 128)],
                                 rhs=ot_s[0][:, ts(n, 512)],
                                 start=False, stop=False)
                nc.tensor.matmul(fp[:], lhsT=wfo_s[1][:, ts(fh, 128)],
                                 rhs=ot_s[1][:, ts(n, 512)],
                                 start=False, stop=True)
                fo = fo_pool.tile([128, 512], F32, tag="fo", name="fo")
                nc.vector.tensor_scalar(fo[:], fp[:], bfe_s[fh][:], None,
                                        op0=mybir.AluOpType.add)
                eng = nc.sync if (fh + n) % 2 == 0 else nc.gpsimd
                eng.dma_start(d_out[ts(fh, 128), ts(n, 512)], fo[:])

    nc.compile()
    return nc


def get_nc():
    if "nc" not in _CACHE:
        _CACHE["nc"] = _build_nc()
    return _CACHE["nc"]


def kernel(x, Wq, bq, Wk, bk, Wv, bv, gamma, Wf, bf, **run_kwargs):
    x = np.asarray(x, np.float32)
    Wq = np.asarray(Wq, np.float32)
    bq = np.asarray(bq, np.float32)
    Wk = np.asarray(Wk, np.float32)
    bk = np.asarray(bk, np.float32)
    Wv = np.asarray(Wv, np.float32)
    bv = np.asarray(bv, np.float32)
    gamma = np.float32(np.asarray(gamma))
    Wf = np.asarray(Wf, np.float32)
    bf = np.asarray(bf, np.float32)

    xf = x.reshape(B, C, N)

    wq4 = np.ascontiguousarray(np.tile(Wq.T, (1, 4)).astype(NP_BF16))   # [256,128]
    wk4 = np.ascontiguousarray(np.tile(Wk.T, (1, 4)).astype(NP_BF16))
    wv = np.ascontiguousarray(Wv.T.astype(NP_BF16))                     # [256,256]
    wfo = np.ascontiguousarray((gamma * Wf[:, :C]).T.astype(NP_BF16))   # [c, f]
    wfx = np.ascontiguousarray(Wf[:, C:].T.astype(np.float32))          # [cx, f]
    bq4 = np.ascontiguousarray(np.tile(bq, 4)[:, None].astype(np.float32))
    bk4 = np.ascontiguousarray(np.tile(bk, 4)[:, None].astype(np.float32))
    bfe = np.ascontiguousarray(
        (bf + gamma * (Wf[:, :C] @ bv))[:, None].astype(np.float32))
    ident = np.eye(128, dtype=NP_BF16)

    in_maps = []
    for core in range(NCORES):
        b, half = core // 2, core % 2
        sl = slice(half * NH, (half + 1) * NH)
        xh = np.ascontiguousarray(xf[b][:, sl])
        other = slice(0, NH) if half == 1 else slice(NH, N)
        xperm = np.concatenate([xf[b][:, sl], xf[b][:, other]], axis=1)
        in_maps.append({
            "xf16": np.ascontiguousarray(xperm.astype(NP_BF16)),
            "xh32": xh,
            "wq4": wq4, "wk4": wk4, "wv": wv, "wfo": wfo, "wfx": wfx,
            "bq4": bq4, "bk4": bk4, "bfe": bfe, "ident": ident,
        })

    nc = get_nc()
    res = run_bass_kernel_spmd(nc, in_maps, list(range(NCORES)), **run_kwargs)

    out = np.empty((B, C, N), np.float32)
    for core in range(NCORES):
        b, half = core // 2, core % 2
        out[b][:, half * NH:(half + 1) * NH] = res.results[core]["out"]
    _CACHE["last_results"] = res
    return out.reshape(B, C, H, W)


if __name__ == "__main__":
    rng = np.random.default_rng(0)
    ins = {
        "x": rng.standard_normal((B, C, H, W), dtype=np.float32),
        "Wq": rng.standard_normal((CK, C), dtype=np.float32) * 0.02,
        "bq": np.zeros(CK, np.float32),
        "Wk": rng.standard_normal((CK, C), dtype=np.float32) * 0.02,
        "bk": np.zeros(CK, np.float32),
        "Wv": rng.standard_normal((C, C), dtype=np.float32) * 0.02,
        "bv": np.zeros(C, np.float32),
        "gamma": np.float32(0.01),
        "Wf": rng.standard_normal((C, 2 * C), dtype=np.float32) * 0.02,
        "bf": np.zeros(C, np.float32),
    }
    out = kernel(**ins)
    print("kernel ran, out shape", out.shape, "finite:", np.isfinite(out).all())

